# revision 23
# baseline (speedup 1.0000x reference)
"""Trainium2 Bass kernel for nn_LinearViolationAdaption.

Per (b,s) row the reference runs up to 51 iterations of
    Ax   = A @ x                    (per-row 512x512 matvec)
    viol = relu(Ax - b); total = sum(viol); active = total >= DELTA
    g    = A^T @ viol               (per-row matvec)
    lr   = ALPHA / (1 + SCALE*g)
    x    = active ? clip(x - lr*g, 0) : x
(the global early-stop never triggers for these inputs, and per-shard
masked updates are mathematically identical anyway).

Sharding: pure data parallel over the 256 (b,s) rows -> 32 rows per core.

Per-core kernel design (compute-bound, PE-ingest roofline):
 - Both A layouts per row in bf16, [128, 4*512]: AT (n on partitions) feeds
   Ax; An (m on partitions) feeds g. 20+20 row-layouts stay SBUF resident,
   the other 24 stream from HBM every iteration (12 x 1MB pairs).
 - Matvec: stationary = vector chunk [128,1], moving = A chunk [128,512]
   bf16 (1 col/cycle). 4 rows share a PSUM bank at partitions {0,32,64,96}
   via explicit tile_position=(0,32j).
 - PSUM rows -> ACT full-bank copy -> SBUF -> DMA partition-gather ->
   dense [16,512] tiles; all elementwise work is dense on DVE/ACT.
 - viol/x go back to partition-major stationary layout via PE transposes.
 - lr*g == (ALPHA - 1/u)/SCALE with u = g*SCALE/ALPHA + 1/ALPHA
   (one ACT affine + accurate DVE reciprocal).
 - x_new = x - mask*min(lr*g, x) reproduces the masked clip exactly.
"""

import numpy as np
import ml_dtypes

import concourse.bass as bass
import concourse.bacc as bacc
import concourse.mybir as mybir
from concourse.tile import TileContext
from concourse.alu_op_type import AluOpType
from concourse.bass_utils import run_bass_kernel_spmd

ALPHA = 0.005
SCALE = 0.001
DELTA = 0.1
ITERS = 51          # MAX_ITER + 1
B, S, M, N = 32, 8, 512, 512
NCORES = 8
ROWS = 32           # rows per core
NQ = ROWS // 4      # 8 quads of 4 rows
F32 = mybir.dt.float32
BF16 = mybir.dt.bfloat16

# rows whose AT / An layout stays resident (core-local indices)
AT_RES_ROWS = [4 * q + j for q in range(6) for j in (0, 1)] + list(range(24, 32))
AN_RES_ROWS = [4 * q + j for q in range(6) for j in (2, 3)] + list(range(24, 32))
# stream pairs, in per-iteration issue order: 6 AT pairs then 6 An pairs
STRM_AT_PAIRS = [(4 * q + 2, 4 * q + 3) for q in range(6)]
STRM_AN_PAIRS = [(4 * q + 0, 4 * q + 1) for q in range(6)]
N_PAIRS = len(STRM_AT_PAIRS) + len(STRM_AN_PAIRS)

UNROLL = 3          # For_i_unrolled factor (51 = 3*17)


def build_nc(iters=ITERS, unroll=UNROLL, phase=3):
    nc = bacc.Bacc(trn_type="TRN2")

    at_res_d = nc.dram_tensor("at_res", [len(AT_RES_ROWS), 128, 2048], BF16, kind="ExternalInput")
    an_res_d = nc.dram_tensor("an_res", [len(AN_RES_ROWS), 128, 2048], BF16, kind="ExternalInput")
    strm_d = nc.dram_tensor("strm", [N_PAIRS, 128, 4096], BF16, kind="ExternalInput")
    x0_d = nc.dram_tensor("x0", [ROWS, 512], F32, kind="ExternalInput")
    b_d = nc.dram_tensor("bmat", [ROWS, 512], F32, kind="ExternalInput")
    sbd_d = nc.dram_tensor("sbd", [ROWS, 1], F32, kind="ExternalInput")
    xp0_d = nc.dram_tensor("xp0", [128, 128], BF16, kind="ExternalInput")
    ident_d = nc.dram_tensor("ident", [16, 16], BF16, kind="ExternalInput")
    xout_d = nc.dram_tensor("xout", [ROWS, 512], F32, kind="ExternalOutput")

    at_res_idx = {r: i for i, r in enumerate(AT_RES_ROWS)}
    an_res_idx = {r: i for i, r in enumerate(AN_RES_ROWS)}

    with TileContext(nc) as tc:
        with (
            tc.tile_pool(name="resA", bufs=1) as resA,       # resident A + consts
            tc.tile_pool(name="strm", bufs=2) as strm,       # streamed A pairs
            tc.tile_pool(name="spill", bufs=1) as spill_p,   # psum->sbuf sparse
            tc.tile_pool(name="dense", bufs=2) as dense_p,   # gathered dense
            tc.tile_pool(name="work", bufs=3) as work_p,     # update temps
            tc.tile_pool(name="small", bufs=2) as small_p,   # t/viol/xbf/masks
            tc.tile_pool(name="state", bufs=1) as state_p,   # x masters
            tc.tile_pool(name="ps_mm", bufs=4, space="PSUM") as ps_mm,
            tc.tile_pool(name="ps_tr", bufs=2, space="PSUM") as ps_tr,
        ):
            # ---- constants / initial state ----
            at_res_t = []
            for i in range(len(AT_RES_ROWS)):
                t = resA.tile([128, 2048], BF16, tag=f"at{i}")
                nc.sync.dma_start(t[:], at_res_d[i])
                at_res_t.append(t)
            an_res_t = []
            for i in range(len(AN_RES_ROWS)):
                t = resA.tile([128, 2048], BF16, tag=f"an{i}")
                nc.sync.dma_start(t[:], an_res_d[i])
                an_res_t.append(t)
            b_t, sbd_t, x_t = [], [], []
            for h in (0, 1):
                bt = resA.tile([16, 512], F32, tag=f"b{h}")
                nc.sync.dma_start(bt[:], b_d[16 * h:16 * h + 16, :])
                b_t.append(bt)
                st = resA.tile([16, 1], F32, tag=f"sbd{h}")
                nc.sync.dma_start(st[:], sbd_d[16 * h:16 * h + 16, :])
                sbd_t.append(st)
                xt = state_p.tile([16, 512], F32, tag=f"x{h}")
                nc.sync.dma_start(xt[:], x0_d[16 * h:16 * h + 16, :])
                x_t.append(xt)
            ident = resA.tile([16, 16], BF16, tag="ident")
            nc.sync.dma_start(ident[:], ident_d[:])
            ubias = resA.tile([16, 1], F32, tag="ubias")
            nc.vector.memset(ubias[:], 1.0 / ALPHA)
            xpart_cur = []
            for h in (0, 1):
                xp = state_p.tile([128, 64], BF16, tag=f"xpart{h}")
                nc.sync.dma_start(xp[:], xp0_d[:, 64 * h:64 * h + 64])
                xpart_cur.append(xp)

            def at_src(r, it_tiles):
                if r in at_res_idx:
                    return at_res_t[at_res_idx[r]], 0
                q = (r - 2) // 4
                return it_tiles[q], 2048 * (r - 4 * q - 2)

            def an_src(r, it_tiles):
                if r in an_res_idx:
                    return an_res_t[an_res_idx[r]], 0
                q = r // 4
                return it_tiles[6 + q], 2048 * (r - 4 * q)

            def mm_quad(q, stat_part, stat_col_of, src_of, it_tiles):
                """16 matvec MMs for quad q into one PSUM bank; returns bank."""
                pa = ps_mm.tile([128, 512], F32, tag="mm")
                for j in range(4):
                    r = 4 * q + j
                    src, off = src_of(r, it_tiles)
                    for c in range(4):
                        nc.tensor.matmul(
                            pa[32 * j:32 * j + 1, :],
                            stat_part[:, stat_col_of(r, c):stat_col_of(r, c) + 1],
                            src[:, off + 512 * c: off + 512 * (c + 1)],
                            start=(c == 0), stop=(c == 3),
                            tile_position=(0, 32 * j),
                        )
                return pa

            def gather(pa, dense_tile, k):
                """PSUM bank -> ACT copy -> sbuf sparse -> DMA gather to dense."""
                sp = spill_p.tile([128, 512], F32, tag="sp")
                for j in range(4):
                    rowap = slice(32 * j, 32 * j + 1)
                    if j % 2 == 0:
                        nc.scalar.copy(sp[rowap, :], pa[rowap, :])
                    else:
                        nc.vector.tensor_copy(sp[rowap, :], pa[rowap, :])
                nc.sync.dma_start(dense_tile[4 * k:4 * k + 4, :], sp[0:128:32, :])

            def transpose4(src_bf, out_ap=None, pool_tag=None):
                """[16,512] bf16 -> [128,64] bf16 partition-major via 4 PE transposes."""
                pt = ps_tr.tile([128, 64], BF16, tag="pt")
                for c in range(4):
                    nc.tensor.transpose(
                        pt[:, 16 * c:16 * c + 16],
                        src_bf[:, 128 * c:128 * (c + 1)],
                        ident[:],
                    )
                if out_ap is None:
                    out = small_p.tile([128, 64], BF16, tag=pool_tag)
                    out_ap = out[:]
                    nc.vector.tensor_copy(out_ap, pt[:])
                    return out
                nc.vector.tensor_copy(out_ap, pt[:])
                return None

            def body(iv):
                # stream this iteration's non-resident A row-pairs
                it_tiles = []
                for p in range(N_PAIRS):
                    t = strm.tile([128, 4096], BF16, tag="strm")
                    nc.sync.dma_start(t[:], strm_d[p])
                    it_tiles.append(t)

                # ---- Ax phase ----
                ax_dense = []
                for h in (0, 1):
                    ad = dense_p.tile([16, 512], F32, tag="dense")
                    for k in range(4):
                        q = 4 * h + k
                        pa = mm_quad(
                            q, xpart_cur[h],
                            lambda r, c, h=h: 16 * c + (r - 16 * h),
                            at_src, it_tiles)
                        gather(pa, ad, k)
                    ax_dense.append(ad)

                if phase == 1:
                    for h in (0, 1):
                        nc.sync.dma_start(xout_d[16 * h:16 * h + 16, :], ax_dense[h][:])
                    return
                # ---- viol / mask / g phase ----
                viol_part, mask_t = [], []
                for h in (0, 1):
                    t_t = small_p.tile([16, 512], F32, tag="t")
                    traw = small_p.tile([16, 1], F32, tag="traw")
                    nc.vector.tensor_tensor(
                        out=t_t[:], in0=ax_dense[h][:], in1=b_t[h][:],
                        op=AluOpType.max)
                    nc.vector.tensor_reduce(
                        out=traw[:], in_=t_t[:],
                        axis=mybir.AxisListType.X, op=AluOpType.add)
                    if phase == 14:
                        continue
                    mk = small_p.tile([16, 1], F32, tag="mask")
                    nc.vector.tensor_tensor(
                        out=mk[:], in0=traw[:], in1=sbd_t[h][:],
                        op=AluOpType.is_ge)
                    mask_t.append(mk)
                    nc.vector.tensor_tensor(
                        out=t_t[:], in0=t_t[:], in1=b_t[h][:],
                        op=AluOpType.subtract)
                    vb = small_p.tile([16, 512], BF16, tag="vb")
                    nc.vector.tensor_copy(vb[:], t_t[:])
                    if phase != 15:
                        viol_part.append(transpose4(vb, pool_tag="vp"))
                if phase in (14, 15):
                    for h in (0, 1):
                        nc.sync.dma_start(xout_d[16 * h:16 * h + 16, :], ax_dense[h][:])
                    return

                g_dense = []
                for h in (0, 1):
                    gd = dense_p.tile([16, 512], F32, tag="dense")
                    for k in range(4):
                        q = 4 * h + k
                        pg = mm_quad(
                            q, viol_part[h],
                            lambda r, c, h=h: 16 * c + (r - 16 * h),
                            an_src, it_tiles)
                        gather(pg, gd, k)
                    g_dense.append(gd)

                if phase == 2:
                    for h in (0, 1):
                        nc.sync.dma_start(xout_d[16 * h:16 * h + 16, :], g_dense[h][:])
                    return
                # ---- update phase ----
                for h in (0, 1):
                    u = work_p.tile([16, 512], F32, tag="wk")
                    nc.scalar.activation(
                        u[:], g_dense[h][:], mybir.ActivationFunctionType.Identity,
                        bias=ubias[:], scale=SCALE / ALPHA)
                    rp = work_p.tile([16, 512], F32, tag="wk")
                    scr = work_p.tile([16, 512], F32, tag="wk")
                    nc.vector.reciprocal_approx_accurate(out=rp[:], in_=u[:], scratch=scr[:])
                    s = work_p.tile([16, 512], F32, tag="wk")
                    nc.vector.tensor_scalar(
                        out=s[:], in0=rp[:],
                        scalar1=-1.0 / SCALE, scalar2=ALPHA / SCALE,
                        op0=AluOpType.mult, op1=AluOpType.add)
                    m2 = work_p.tile([16, 512], F32, tag="wk")
                    nc.vector.tensor_tensor(
                        out=m2[:], in0=s[:], in1=x_t[h][:], op=AluOpType.min)
                    md = work_p.tile([16, 512], F32, tag="wk")
                    nc.vector.tensor_scalar(
                        out=md[:], in0=m2[:], scalar1=mask_t[h][0:16, 0:1],
                        scalar2=None, op0=AluOpType.mult)
                    nc.vector.tensor_tensor(
                        out=x_t[h][:], in0=x_t[h][:], in1=md[:],
                        op=AluOpType.subtract)
                    xb = small_p.tile([16, 512], BF16, tag="xbf")
                    nc.vector.tensor_copy(xb[:], x_t[h][:])
                    transpose4(xb, out_ap=xpart_cur[h][:])

            if unroll == 0:
                for _ in range(iters):
                    body(0)
            else:
                tc.For_i_unrolled(0, iters, 1, body, max_unroll=unroll)

            for h in (0, 1):
                nc.sync.dma_start(xout_d[16 * h:16 * h + 16, :], x_t[h][:])

    nc.compile()
    return nc


def _prep_core(xs, As, bs):
    """Host-side per-core input prep. xs [32,512] f32, As [32,512,512] f32, bs [32,512] f32."""
    bf = ml_dtypes.bfloat16
    # AT[r, p, c*512+m] = A[r, m, c*128+p]  (n on partitions)
    AT = np.ascontiguousarray(
        As.reshape(ROWS, 512, 4, 128).transpose(0, 3, 2, 1).reshape(ROWS, 128, 2048)
    ).astype(bf)
    # An[r, p, c*512+n] = A[r, c*128+p, n]  (m on partitions)
    An = np.ascontiguousarray(
        As.reshape(ROWS, 4, 128, 512).transpose(0, 2, 1, 3).reshape(ROWS, 128, 2048)
    ).astype(bf)
    strm = np.empty((N_PAIRS, 128, 4096), dtype=bf)
    for p, (r0, r1) in enumerate(STRM_AT_PAIRS):
        strm[p, :, :2048] = AT[r0]
        strm[p, :, 2048:] = AT[r1]
    for p, (r0, r1) in enumerate(STRM_AN_PAIRS):
        strm[6 + p, :, :2048] = An[r0]
        strm[6 + p, :, 2048:] = An[r1]
    # xp0[p, h*64 + c*16 + r] = x[16h+r, c*128+p]
    xp0 = np.ascontiguousarray(
        xs.reshape(2, 16, 4, 128).transpose(3, 0, 2, 1).reshape(128, 128)
    ).astype(bf)
    sbd = (bs.astype(np.float32).sum(axis=1, keepdims=True) + np.float32(DELTA)).astype(np.float32)
    return {
        "at_res": np.ascontiguousarray(AT[AT_RES_ROWS]),
        "an_res": np.ascontiguousarray(An[AN_RES_ROWS]),
        "strm": strm,
        "x0": np.ascontiguousarray(xs.astype(np.float32)),
        "bmat": np.ascontiguousarray(bs.astype(np.float32)),
        "sbd": sbd,
        "xp0": xp0,
        "ident": np.eye(16, dtype=bf),
    }


_NC_CACHE = {}


def _get_nc(iters=ITERS, unroll=UNROLL, phase=3):
    key = (iters, unroll, phase)
    if key not in _NC_CACHE:
        _NC_CACHE[key] = build_nc(iters, unroll, phase)
    return _NC_CACHE[key]


def kernel(x, A, b, _iters=ITERS, _unroll=UNROLL, _trace=False, _phase=3):
    x = np.asarray(x, dtype=np.float32).reshape(B * S, N)
    A = np.asarray(A, dtype=np.float32).reshape(B * S, M, N)
    b = np.asarray(b, dtype=np.float32).reshape(B * S, M)

    nc = _get_nc(_iters, _unroll, _phase)
    in_maps = []
    for c in range(NCORES):
        rows = slice(ROWS * c, ROWS * (c + 1))
        in_maps.append(_prep_core(x[rows], A[rows], b[rows]))
    res = run_bass_kernel_spmd(nc, in_maps, core_ids=list(range(NCORES)), trace=_trace)
    out = np.concatenate([r["xout"] for r in res.results], axis=0)
    out = out.reshape(B, S, N).astype(np.float32)
    if _trace:
        kernel.last_results = res
    return out
